# revision 2
# baseline (speedup 1.0000x reference)
"""Trainium2 Bass kernel for nn_AlphaModel (gnn_message_passing).

Strategy: host-side bucket sort of edges by relation (layout-only work, part
of sharding), 8 relations per core across 8 cores. Within a core, edges are
arranged in a grouped layout [126, W] = 42 groups x 3 components so that:
  - the relation-indexed 3x3 matvec becomes a block-diagonal PE matmul with
    per-tile weights (M arriving as DMA'd data),
  - softmax is normalized by accumulating -ln(sum exp) into the scores PSUM
    through a broadcast matmul, so ACT exp directly yields normalized child,
  - per-edge component sums (entropy sums, dot products) are selector-matrix
    PE matmuls, packed two tiles per PSUM bank at partition offsets 0 / 64,
  - per-edge scale factors broadcast back to components via PE,
  - beta becomes a per-partition constant vector.
All reciprocals/rsqrts are computed as exp(-k*ln(x)) so ACT uses a single
table set (natural_log_exp_and_others).

kernel(**inputs) takes FULL unsharded inputs and returns the FULL output.
"""

import os
import sys
import types
import numpy as np

W = 512            # edges per group-run (= matmul moving free dim)
G = 42             # groups per tile (42*3 = 126 partitions)
TILE_E = G * W     # edges per tile
SUPER = 2          # tiles per supertile (sums packed at partition 0 / 64)
ST_E = SUPER * TILE_E
N_CORES = 8
N_RELS = 64

LAST_EXEC_TIME_NS = None
_NC_CACHE = {}


def _ensure_ntff_hook():
    """Inject the missing antenv.axon_hooks module and register the NTFF
    profile hook so run_bass_kernel_spmd(trace=True) can report exec_time_ns."""
    try:
        if "antenv.axon_hooks" not in sys.modules:
            mod = types.ModuleType("antenv.axon_hooks")
            mod._hook = None
            mod.set_axon_ntff_profile_hook = lambda h: setattr(mod, "_hook", h)
            mod.get_axon_ntff_profile_hook = lambda: mod._hook
            sys.modules["antenv.axon_hooks"] = mod
            import antenv
            antenv.axon_hooks = mod
        mod = sys.modules["antenv.axon_hooks"]
        if mod.get_axon_ntff_profile_hook() is None:
            from trn_agent_boot.trn_boot import _ntff_profile_via_ctypes
            mod.set_axon_ntff_profile_hook(
                _ntff_profile_via_ctypes("/opt/axon/libaxon_pjrt.so"))
        return mod.get_axon_ntff_profile_hook() is not None
    except Exception:
        return False


# --------------------------------------------------------------------------
# Host-side plan: bucket, shard, pad, group.
# --------------------------------------------------------------------------

def build_plan(rels):
    """Returns per-core edge index arrays (with -1 for padding) and the
    relation of every (tile, group)."""
    rels = np.asarray(rels)
    order = np.argsort(rels, kind="stable")
    counts = np.bincount(rels.astype(np.int64), minlength=N_RELS)
    starts = np.concatenate([[0], np.cumsum(counts)])

    # LPT assignment of relations to cores (balanced edge totals).
    core_rels = [[] for _ in range(N_CORES)]
    core_load = np.zeros(N_CORES, dtype=np.int64)
    for r in np.argsort(counts)[::-1]:
        c = int(np.argmin(core_load))
        core_rels[c].append(int(r))
        core_load[c] += int(counts[r])

    # Per-core: concat segments, each padded to a multiple of W.
    core_idx = []      # padded edge-index arrays (-1 = dummy)
    core_grel = []     # relation id per group-run
    for c in range(N_CORES):
        pieces, grels = [], []
        for r in core_rels[c]:
            n = int(counts[r])
            if n == 0:
                continue
            seg = order[starts[r]:starts[r] + n]
            pad = (-n) % W
            if pad:
                seg = np.concatenate([seg, np.full(pad, -1, dtype=seg.dtype)])
            pieces.append(seg)
            grels.extend([r] * (len(seg) // W))
        idx = (np.concatenate(pieces) if pieces
               else np.zeros(0, dtype=np.int64))
        core_idx.append(idx)
        core_grel.append(grels)

    # Pad every core to a common multiple of ST_E.
    max_n = max(max(len(i) for i in core_idx), ST_E)
    total = -(-max_n // ST_E) * ST_E
    T = total // TILE_E
    for c in range(N_CORES):
        pad = total - len(core_idx[c])
        if pad:
            core_idx[c] = np.concatenate(
                [core_idx[c], np.full(pad, -1, dtype=np.int64)])
            fill_rel = core_grel[c][-1] if core_grel[c] else 0
            core_grel[c].extend([fill_rel] * (pad // W))
        core_grel[c] = np.asarray(core_grel[c], dtype=np.int64).reshape(T, G)

    return core_idx, core_grel, T


def _group_planes(arr_core, S):
    """[N,3] edge-major -> [S, 126, SUPER, W] grouped planes."""
    a = arr_core.reshape(S, SUPER, G, W, 3)          # s, j, g, w, c
    return np.ascontiguousarray(
        a.transpose(0, 2, 4, 1, 3).reshape(S, 126, SUPER, W))


def _ungroup_planes(out_core, S):
    """[S, 126, SUPER, W] -> [N, 3]."""
    a = out_core.reshape(S, G, 3, SUPER, W)          # s, g, c, j, w
    return np.ascontiguousarray(
        a.transpose(0, 3, 1, 4, 2).reshape(S * SUPER * G * W, 3))


# index templates for block-diagonal weight assembly
_g = np.arange(G)[:, None, None]
_i = np.arange(3)[None, :, None]
_j = np.arange(3)[None, None, :]
_BD_ROW = ((3 * _g + _j) * np.ones((G, 3, 3), np.int64)).astype(np.int64)
_BD_COL = ((3 * _g + _i) * np.ones((G, 3, 3), np.int64)).astype(np.int64)


def build_core_inputs(prnt, child, M, beta, idx, grel, T):
    """Per-core device input arrays."""
    S = T // SUPER
    safe = np.maximum(idx, 0)
    p = prnt[safe].astype(np.float32)
    c = child[safe].astype(np.float32)
    bad = idx < 0
    if bad.any():
        p[bad] = 0.5
        c[bad] = 0.5
    pin = _group_planes(p, S)
    cin = _group_planes(c, S)

    # Block-diagonal weights per tile: wts[t, 3g+j, 3g+i] = M[rel(t,g), i, j]
    wts = np.zeros((T, 126, 126), dtype=np.float32)
    wts[:, _BD_ROW, _BD_COL] = M[grel].astype(np.float32)

    # beta tables: btbl[3g+c, t] = beta[rel(t,g), c]
    Bt = beta[grel].astype(np.float32)               # [T, G, 3]
    btbl = np.ascontiguousarray(Bt.transpose(1, 2, 0).reshape(126, T))
    ombtbl = np.ascontiguousarray(1.0 - btbl)

    # selector [126 -> 42]: sel[3g+c, g] = 1
    sel = np.zeros((126, G), dtype=np.float32)
    sel[np.arange(126), np.arange(126) // 3] = 1.0
    # broadcast [42 -> 126] placed at both partition blocks 0 and 64
    bc3x = np.zeros((128, 126), dtype=np.float32)
    bc3x[0:G, :] = sel.T
    bc3x[64:64 + G, :] = sel.T
    bc3nx = np.ascontiguousarray(-bc3x)

    return {"pin": pin, "cin": cin, "wts": wts, "btbl": btbl,
            "ombtbl": ombtbl, "sel": sel, "bc3x": bc3x, "bc3nx": bc3nx}


# --------------------------------------------------------------------------
# Device kernel
# --------------------------------------------------------------------------

def build_nc(S, T, eps, sf):
    import concourse.bacc as bacc
    import concourse.tile as tile
    from concourse import mybir

    f32 = mybir.dt.float32
    Alu = mybir.AluOpType
    Act = mybir.ActivationFunctionType
    NP = 106   # used rows of packed per-edge tiles (0:42 and 64:106)

    nc = bacc.Bacc("TRN2", target_bir_lowering=False, debug=False,
                   num_devices=N_CORES)
    pin = nc.dram_tensor("pin", [S, 126, SUPER, W], f32, kind="ExternalInput").ap()
    cin = nc.dram_tensor("cin", [S, 126, SUPER, W], f32, kind="ExternalInput").ap()
    wts = nc.dram_tensor("wts", [T, 126, 126], f32, kind="ExternalInput").ap()
    btbl = nc.dram_tensor("btbl", [126, T], f32, kind="ExternalInput").ap()
    ombtbl = nc.dram_tensor("ombtbl", [126, T], f32, kind="ExternalInput").ap()
    sel = nc.dram_tensor("sel", [126, G], f32, kind="ExternalInput").ap()
    bc3x = nc.dram_tensor("bc3x", [128, 126], f32, kind="ExternalInput").ap()
    bc3nx = nc.dram_tensor("bc3nx", [128, 126], f32, kind="ExternalInput").ap()
    outp = nc.dram_tensor("out", [S, 126, SUPER, W], f32, kind="ExternalOutput").ap()

    def blk(j):
        return slice(64 * j, 64 * j + G)

    with tile.TileContext(nc) as tc:
        with (
            tc.tile_pool(name="consts", bufs=1) as consts,
            tc.tile_pool(name="wtp", bufs=3) as wtp,
            tc.tile_pool(name="planes", bufs=2) as planes,
            tc.tile_pool(name="small", bufs=2) as small,
            tc.tile_pool(name="ps_big", bufs=2, space="PSUM") as ps_big,
            tc.tile_pool(name="ps_sum", bufs=6, space="PSUM") as ps_sum,
        ):
            sel_sb = consts.tile([126, G], f32)
            nc.gpsimd.dma_start(out=sel_sb[:], in_=sel)
            bc3x_sb = consts.tile([128, 126], f32)
            nc.gpsimd.dma_start(out=bc3x_sb[:], in_=bc3x)
            bc3nx_sb = consts.tile([128, 126], f32)
            nc.gpsimd.dma_start(out=bc3nx_sb[:], in_=bc3nx)
            b_sb = consts.tile([126, T], f32)
            nc.gpsimd.dma_start(out=b_sb[:], in_=btbl)
            omb_sb = consts.tile([126, T], f32)
            nc.gpsimd.dma_start(out=omb_sb[:], in_=ombtbl)

            for s in range(S):
                P3 = planes.tile([126, SUPER, W], f32, tag="P3")
                nc.gpsimd.dma_start(out=P3[:], in_=pin[s])
                C3 = planes.tile([126, SUPER, W], f32, tag="C3")
                nc.gpsimd.dma_start(out=C3[:], in_=cin[s])

                # --- scores, softmax denominator --------------------------
                E3 = planes.tile([126, SUPER, W], f32, tag="E3")
                ze2 = ps_sum.tile([128, W], f32, tag="sums")
                scs = []
                for j in range(SUPER):
                    t = s * SUPER + j
                    wt = wtp.tile([126, 126], f32, tag="wt")
                    nc.gpsimd.dma_start(out=wt[:], in_=wts[t])
                    sc = ps_big.tile([126, W], f32, tag="big")
                    nc.tensor.matmul(sc[:], wt[:], C3[:, j, :],
                                     start=True, stop=False,
                                     skip_group_check=True)
                    scs.append(sc)
                    nc.scalar.activation(E3[:, j, :], sc[:], Act.Exp)
                    nc.tensor.matmul(ze2[blk(j), :], sel_sb[:], E3[:, j, :],
                                     skip_group_check=True)

                lgZe = small.tile([128, W], f32, tag="lgZe")
                nc.scalar.activation(lgZe[:NP], ze2[:NP], Act.Ln)

                # normalized child: ch = exp(scores - ln(Ze))
                ch3 = planes.tile([126, SUPER, W], f32, tag="ch3")
                for j in range(SUPER):
                    nc.tensor.matmul(scs[j][:], bc3nx_sb[blk(j), :],
                                     lgZe[blk(j), :],
                                     start=False, stop=True,
                                     skip_group_check=True)
                    nc.scalar.activation(ch3[:, j, :], scs[j][:], Act.Exp)

                # --- blend: A = (1-b)*P + b*ch ----------------------------
                A3 = planes.tile([126, SUPER, W], f32, tag="A3")
                for j in range(SUPER):
                    t = s * SUPER + j
                    nc.vector.tensor_scalar_mul(
                        out=A3[:, j, :], in0=P3[:, j, :],
                        scalar1=omb_sb[:, t:t + 1])
                    nc.vector.scalar_tensor_tensor(
                        out=A3[:, j, :], in0=ch3[:, j, :],
                        scalar=b_sb[:, t:t + 1], in1=A3[:, j, :],
                        op0=Alu.mult, op1=Alu.add)

                # --- z path (entropy) -------------------------------------
                Y3 = planes.tile([126, SUPER, W], f32, tag="Y3")
                nc.vector.tensor_tensor(Y3[:], P3[:], ch3[:], Alu.add)
                nc.vector.tensor_scalar(out=Y3[:], in0=Y3[:],
                                        scalar1=float(eps), scalar2=None,
                                        op0=Alu.max)
                L3 = planes.tile([126, SUPER, W], f32, tag="L3")
                nc.scalar.activation(L3[:], Y3[:], Act.Ln)
                zs2 = ps_sum.tile([128, W], f32, tag="sums")
                t2 = ps_sum.tile([128, W], f32, tag="sums")
                for j in range(SUPER):
                    nc.tensor.matmul(zs2[blk(j), :], sel_sb[:], Y3[:, j, :],
                                     skip_group_check=True)
                nc.vector.tensor_tensor(L3[:], Y3[:], L3[:], Alu.mult)
                for j in range(SUPER):
                    nc.tensor.matmul(t2[blk(j), :], sel_sb[:], L3[:, j, :],
                                     skip_group_check=True)

                # --- cosine path ------------------------------------------
                pn2 = ps_sum.tile([128, W], f32, tag="sums")
                en2 = ps_sum.tile([128, W], f32, tag="sums")
                dot2 = ps_sum.tile([128, W], f32, tag="sums")
                Q3 = planes.tile([126, SUPER, W], f32, tag="Q3")
                nc.scalar.activation(Q3[:], P3[:], Act.Square)
                for j in range(SUPER):
                    nc.tensor.matmul(pn2[blk(j), :], sel_sb[:], Q3[:, j, :],
                                     skip_group_check=True)
                Q3b = planes.tile([126, SUPER, W], f32, tag="Q3")
                nc.scalar.activation(Q3b[:], ch3[:], Act.Square)
                for j in range(SUPER):
                    nc.tensor.matmul(en2[blk(j), :], sel_sb[:], Q3b[:, j, :],
                                     skip_group_check=True)
                Q3c = planes.tile([126, SUPER, W], f32, tag="Q3")
                nc.vector.tensor_tensor(Q3c[:], P3[:], ch3[:], Alu.mult)
                for j in range(SUPER):
                    nc.tensor.matmul(dot2[blk(j), :], sel_sb[:], Q3c[:, j, :],
                                     skip_group_check=True)

                # --- per-edge scalar chain (packed [0:106] rows) ----------
                LZ = small.tile([128, W], f32, tag="LZ")
                nc.scalar.activation(LZ[:NP], zs2[:NP], Act.Ln)
                RZ = small.tile([128, W], f32, tag="RZ")
                nc.scalar.activation(RZ[:NP], LZ[:NP], Act.Exp, scale=-1.0)
                H = small.tile([128, W], f32, tag="H")
                nc.vector.tensor_tensor(H[:NP], t2[:NP], RZ[:NP], Alu.mult)
                nc.vector.tensor_tensor(H[:NP], LZ[:NP], H[:NP], Alu.subtract)
                nc.scalar.activation(H[:NP], H[:NP], Act.Ln)
                RH = small.tile([128, W], f32, tag="RH")
                nc.scalar.activation(RH[:NP], H[:NP], Act.Exp, scale=-1.0)

                lg = small.tile([128, W], f32, tag="lg")
                nc.scalar.activation(lg[:NP], pn2[:NP], Act.Ln)
                rs = small.tile([128, W], f32, tag="rs")
                nc.scalar.activation(rs[:NP], en2[:NP], Act.Ln)
                nc.vector.tensor_tensor(lg[:NP], lg[:NP], rs[:NP], Alu.add)
                nc.scalar.activation(rs[:NP], lg[:NP], Act.Exp, scale=-0.5)

                Sc = small.tile([128, W], f32, tag="Sc")
                nc.vector.scalar_tensor_tensor(
                    out=Sc[:NP], in0=dot2[:NP], scalar=float(sf), in1=rs[:NP],
                    op0=Alu.mult, op1=Alu.mult)
                nc.vector.scalar_tensor_tensor(
                    out=Sc[:NP], in0=Sc[:NP], scalar=float(1.1 * sf),
                    in1=RH[:NP], op0=Alu.add, op1=Alu.mult)

                # --- scale broadcast + output -----------------------------
                O3 = planes.tile([126, SUPER, W], f32, tag="O3")
                for j in range(SUPER):
                    sb = ps_big.tile([126, W], f32, tag="big")
                    nc.tensor.matmul(sb[:], bc3x_sb[blk(j), :], Sc[blk(j), :],
                                     skip_group_check=True)
                    nc.vector.tensor_tensor(O3[:, j, :], A3[:, j, :], sb[:],
                                            Alu.mult)
                nc.gpsimd.dma_start(out=outp[s], in_=O3[:])

    nc.compile()
    return nc


# --------------------------------------------------------------------------
# Entry point
# --------------------------------------------------------------------------

def kernel(var_sfx=None, prnt_probs=None, child_probs=None, rels=None,
           M=None, beta=None, z_epsilon=None, scale_factor=None, **_):
    global LAST_EXEC_TIME_NS
    from concourse.bass_utils import run_bass_kernel_spmd

    prnt = np.asarray(prnt_probs, dtype=np.float32)
    child = np.asarray(child_probs, dtype=np.float32)
    rels_np = np.asarray(rels)
    M_np = np.asarray(M, dtype=np.float32)
    beta_np = np.asarray(beta, dtype=np.float32)
    eps = float(np.asarray(z_epsilon))
    sf = float(np.asarray(scale_factor))
    E = prnt.shape[0]

    core_idx, core_grel, T = build_plan(rels_np)
    S = T // SUPER

    in_maps = []
    for c in range(N_CORES):
        in_maps.append(build_core_inputs(
            prnt, child, M_np, beta_np, core_idx[c], core_grel[c], T))

    key = (S, T, eps, sf)
    if key not in _NC_CACHE:
        _NC_CACHE[key] = build_nc(S, T, eps, sf)
    nc = _NC_CACHE[key]

    trace = os.environ.get("BASS_KERNEL_TRACE", "0") == "1"
    if trace:
        trace = _ensure_ntff_hook()
    r = run_bass_kernel_spmd(nc, in_maps, core_ids=list(range(N_CORES)),
                             trace=trace)
    if trace:
        LAST_EXEC_TIME_NS = r.exec_time_ns

    out = np.empty((E, 3), dtype=np.float32)
    for c in range(N_CORES):
        o = _ungroup_planes(r.results[c]["out"], S)
        idx = core_idx[c]
        valid = idx >= 0
        out[idx[valid]] = o[valid]
    return out
